# revision 33
# baseline (speedup 1.0000x reference)
"""MoE router kernel for Trainium2 (raw Bass), 8-core data-parallel.

Problem (hardcoded shapes): hidden_states [4, 8192, 2048] f32, active_mask
[4, 8192] bool, uniform [4, 8192] f32, W [2048] f32, b () f32.

reference = router scores (matvec over D=2048) -> gumbel-noised global
top-k boolean mask + scalar aux loss.

Device work (the memory-bound part, 256 MiB read): scores = hs @ W,
sharded data-parallel over the 32768 tokens across 8 NeuronCores (4096
tokens / 32 MiB each), W replicated.  Per core: a tapered stream of
input DMAs (1 MiB chunks at the edges, 2 MiB steady state, dual HWDGE
rings, 16 MiB block-ring buffer) and 32 fused multiply+reduce ops on
DVE (scalar_tensor_tensor with accum_out) -> 4096 scores.  Measured
~89 us/core steady state (~375 GB/s/core effective), at the HBM-per-
core roofline.

Host work (O(B*S) = 32768 elements, negligible): gumbel noise, global
top-k threshold + mask, aux losses -- done with jax-on-CPU when
available (bit-identical op sequence to the reference), numpy fallback.
"""

import os
import sys

import numpy as np

for _p in ("/opt/trn_rl_repo", os.path.expanduser("~/.axon_site/_ro/trn_rl_repo")):
    if os.path.isdir(_p) and _p not in sys.path:
        sys.path.append(_p)

import concourse.bass as bass
import concourse.mybir as mybir
from concourse.bass_utils import run_bass_kernel_spmd

B, S, D = 4, 8192, 2048
T = B * S                  # 32768 tokens
NCORES = 8
TC = T // NCORES           # 4096 tokens per core
P = 128                    # SBUF partitions
NCOL = TC // P             # 32 score columns (128-token / 1 MiB blocks)
NBUFB = 16                 # h-buffer ring size in blocks (16 MiB)
# tapered DMA chunk schedule, in blocks: small first chunks so DVE starts
# sooner, small last chunks so the drain tail is short; 2 MiB steady-state
BLOCKS_PER_CHUNK = [1, 1] + [2] * 14 + [1, 1]  # sums to NCOL

CAPACITY = 0.55
TEMPERATURE = 1.0
LB_W, Z_W, ENT_W = 0.01, 1e-4, 1e-3

_BUILT = None
last_exec_time_ns = None
last_trace_path = None


def _build_bass(reps: int = 1):
    """Raw-bass SPMD program for one core: scores[t] = sum_d hs[t,d] * w[d].

    reps > 1 repeats the whole pipeline back-to-back inside the NEFF
    (identical work each rep) — used only for wall-clock benchmarking.

    Layout: token t = p*NCOL + c maps to partition p, 128-token (1 MiB)
    block c; blocks are grouped into DMA chunks per BLOCKS_PER_CHUNK
    (tapered: 1 MiB at the edges, 2 MiB steady state) landing in an
    NBUFB-block SBUF ring, then one fused multiply+reduce op
    (TensorScalarPtr w/ accum_out) on DVE per block produces one column
    of ssb [128, NCOL].

    Input DMAs alternate between the SP and ACT HWDGE rings so descriptor
    generation/completion overheads of consecutive transfers overlap; the
    tiny score store runs on GPSIMD (SWDGE) so the input rings never
    stall; the W broadcast heads the ACT ring while SP's chunk 0 (DVE's
    first need) starts in parallel.

    (TileContext is unusable with this compiler build: its end-of-kernel
    barrier emits multi-wait instructions that walrus rejects with "Too
    many sync wait commands", so synchronization is explicit here.)
    """
    blocks = BLOCKS_PER_CHUNK
    nchunk = len(blocks)
    gb_start = [0]
    for k in blocks[:-1]:
        gb_start.append(gb_start[-1] + k)
    # no chunk may wrap the block ring, and reps must realign the ring
    assert sum(blocks) == NCOL and NCOL % NBUFB == 0
    for s, k in zip(gb_start, blocks):
        assert (s % NBUFB) + k <= NBUFB, (s, k)

    nc = bass.Bass(trn_type="TRN2")
    hs = nc.dram_tensor("hs", [TC, D], mybir.dt.float32, kind="ExternalInput")
    w = nc.dram_tensor("w", [D], mybir.dt.float32, kind="ExternalInput")
    scores = nc.dram_tensor("scores", [TC], mybir.dt.float32, kind="ExternalOutput")

    # token t = p*NCOL + c: per-partition contiguous input runs and a
    # contiguous [128, NCOL] score store.
    hs_rr = hs[:].rearrange("(p c) d -> p c d", p=P)     # [128, NCOL, D]
    scores_r = scores[:].rearrange("(p c) -> p c", p=P)  # [128, NCOL]

    rings = [[i for i in range(nchunk) if i % 2 == k] for k in range(2)]
    ring_of = {i: i % 2 for i in range(nchunk)}
    ring_ord = {}
    for k in range(2):
        for o, i in enumerate(rings[k]):
            ring_ord[i] = o
    RING_INC = [16 * len(rings[k]) for k in range(2)]
    # each ring carries half the W broadcast up front (completes in half
    # the time and overlaps the first input chunk)
    V_INC = NCOL  # v_sem counts completed blocks

    with (
        nc.sbuf_tensor([P, D], mybir.dt.float32) as wb,
        nc.sbuf_tensor([P, NBUFB * D], mybir.dt.float32) as hbuf,
        # double-buffered by rep parity so rep r+1's muls never wait on
        # rep r's out-DMA (reps>1 benchmarking only; reps=1 uses half)
        nc.sbuf_tensor([P, 2 * NCOL], mybir.dt.float32) as ssb,
        nc.sbuf_tensor([P, 1], mybir.dt.float32) as dummy,
        nc.semaphore() as sem_sp,
        nc.semaphore() as sem_act,
        nc.semaphore() as sem_pool,
        nc.semaphore() as v_sem,
        # gp's explicit sem_pool wait already proves the score store
        # landed; skip GPSIMD's expensive SWDGE dge_drain at the exit
        # barrier.
        nc.Block(no_gpsimd_drain=True) as block,
    ):
        ring_sems = [sem_sp, sem_act]

        def ssb_half(r):
            base = (r % 2) * NCOL
            return ssb[:, base:base + NCOL]

        def input_stream(eng, k):
            sem = ring_sems[k]
            half = D // 2
            lo, hi = k * half, (k + 1) * half
            eng.dma_start(
                wb[:, lo:hi], w[:][lo:hi].unsqueeze(0).broadcast_to([P, half])
            ).then_inc(sem, 16)
            for r in range(reps):
                for i in rings[k]:
                    kblk = blocks[i]
                    gb = r * NCOL + gb_start[i]  # global block index
                    # ring slot gb%NBUFB is free once the mul of block
                    # gb+kblk-NBUFB has run (v_sem counts muls/blocks)
                    free_at = gb + kblk - NBUFB
                    if free_at > 0:
                        eng.wait_ge(v_sem, free_at)
                    s = gb % NBUFB
                    eng.dma_start(
                        hbuf[:, s * D:(s + kblk) * D],
                        hs_rr[:, gb_start[i]:gb_start[i] + kblk, :],
                    ).then_inc(sem, 16)

        @block.sync
        def _(sync):
            # half of W, then even chunks
            input_stream(sync, 0)

        @block.scalar
        def _(scalar):
            # other half of W, then odd chunks
            input_stream(scalar, 1)

        @block.gpsimd
        def _(gp):
            # tiny per-rep score store
            for r in range(reps):
                gp.wait_ge(v_sem, (r + 1) * V_INC)
                gp.dma_start(scores_r, ssb_half(r)).then_inc(sem_pool, 16)
            gp.wait_ge(sem_pool, 16 * reps)  # final out-DMA landed

        @block.vector
        def _(vector):
            for r in range(reps):
                if r == 0:
                    # ACT's wb half; SP's half is subsumed by chunk 0's wait
                    vector.wait_ge(sem_act, 16)
                elif r >= 2:
                    # rep r-2's out-DMA done (WAR on this parity's ssb)
                    vector.wait_ge(sem_pool, 16 * (r - 1))
                for i in range(nchunk):
                    k = ring_of[i]
                    vector.wait_ge(
                        ring_sems[k],
                        r * RING_INC[k] + 16 + 16 * (ring_ord[i] + 1),
                    )
                    for b in range(blocks[i]):
                        col = gb_start[i] + b
                        s = (r * NCOL + col) % NBUFB
                        nc.vector.scalar_tensor_tensor(
                            dummy[:].broadcast_to([P, D]),
                            hbuf[:, s * D:(s + 1) * D],
                            1.0,
                            wb[:],
                            op0=mybir.AluOpType.bypass,
                            op1=mybir.AluOpType.mult,
                            accum_out=ssb_half(r)[:, col:col + 1],
                        ).then_inc(v_sem, 1)
    return nc


def _device_scores(hidden_states, W):
    """Run the sharded matvec on 8 NeuronCores; returns scores [T] f32."""
    global _BUILT, last_exec_time_ns, last_trace_path
    if _BUILT is None:
        _BUILT = _build_bass()
    nc = _BUILT
    hs_flat = np.ascontiguousarray(
        np.asarray(hidden_states, dtype=np.float32).reshape(T, D)
    )
    w_np = np.ascontiguousarray(np.asarray(W, dtype=np.float32).reshape(D))
    in_maps = [
        {"hs": hs_flat[c * TC:(c + 1) * TC], "w": w_np} for c in range(NCORES)
    ]
    res = run_bass_kernel_spmd(nc, in_maps, core_ids=list(range(NCORES)))
    last_exec_time_ns = res.exec_time_ns
    if res.instructions_and_trace is not None:
        last_trace_path = res.instructions_and_trace[1]
    return np.concatenate([res.results[c]["scores"] for c in range(NCORES)])


def _postprocess_np(scores_flat, active_mask, uniform, b):
    """Numpy fallback for the O(T) post-processing (f32-faithful)."""
    f32 = np.float32
    scores = scores_flat.reshape(B, S) + f32(b)
    scores = np.where(active_mask, scores, f32(-1e9)).astype(np.float32)
    k = max(1, min(int(CAPACITY * T + 0.5), T))
    u = np.clip(uniform.astype(np.float32), f32(1e-6), f32(1.0 - 1e-6))
    gumbel = np.clip(
        -np.log(-np.log(u) + f32(1e-6)), f32(-10.0), f32(10.0)
    ).astype(np.float32)
    noisy = (scores + gumbel * f32(TEMPERATURE)).astype(np.float32)
    flat = noisy.reshape(-1)
    kth = np.partition(flat, T - k)[T - k]
    mask_flat = flat > kth
    n_gt = int(mask_flat.sum())
    if n_gt < k:  # fill ties at the threshold in index order (lax.top_k is stable)
        ties = np.nonzero(flat == kth)[0]
        mask_flat[ties[: k - n_gt]] = True
    ffn_mask = mask_flat.reshape(B, S) & active_mask

    with np.errstate(over="ignore"):
        probs = np.where(
            scores >= 0,
            f32(1.0) / (f32(1.0) + np.exp(-scores)),
            np.exp(scores) / (f32(1.0) + np.exp(scores)),
        ).astype(np.float32)
    lb_loss = (probs.mean(dtype=np.float32) - f32(CAPACITY)) ** 2
    z_loss = (scores.astype(np.float32) ** 2).mean(dtype=np.float32)
    p = np.clip(probs, f32(1e-4), f32(1.0 - 1e-4))
    entropy = -(
        p * np.log(p) + (f32(1.0) - p) * np.log(f32(1.0) - p)
    ).mean(dtype=np.float32)
    aux = f32(LB_W) * lb_loss + f32(Z_W) * z_loss + f32(ENT_W) * (-entropy)
    return np.asarray(ffn_mask, dtype=bool), np.float32(aux)


def _postprocess_jax(scores_flat, active_mask, uniform, b):
    """Reference-identical op sequence on jax CPU (given device scores)."""
    import jax
    import jax.numpy as jnp

    cpu = jax.devices("cpu")[0]
    with jax.default_device(cpu):
        scores = jnp.asarray(scores_flat, dtype=jnp.float32).reshape(B, S) + jnp.asarray(
            b, dtype=jnp.float32
        )
        am = jnp.asarray(active_mask)
        scores = jnp.where(am, scores, jnp.float32(-1e9))
        k = max(1, min(int(CAPACITY * T + 0.5), T))
        u = jnp.clip(jnp.asarray(uniform, dtype=jnp.float32), 1e-6, 1.0 - 1e-6)
        gumbel = jnp.clip(-jnp.log(-jnp.log(u) + 1e-6), -10.0, 10.0)
        noisy = scores + gumbel * TEMPERATURE
        flat = noisy.reshape(-1)
        _, idx = jax.lax.top_k(flat, k)
        ffn_mask = jnp.zeros((T,), dtype=bool).at[idx].set(True)
        ffn_mask = ffn_mask.reshape(B, S) & am

        probs = jax.nn.sigmoid(scores)
        lb_loss = (probs.mean() - jnp.float32(CAPACITY)) ** 2
        z_loss = (scores ** 2).mean()
        p = jnp.clip(probs, 1e-4, 1.0 - 1e-4)
        entropy = -(p * jnp.log(p) + (1.0 - p) * jnp.log(1.0 - p)).mean()
        aux = LB_W * lb_loss + Z_W * z_loss + ENT_W * (-entropy)
        return np.asarray(ffn_mask), np.asarray(aux, dtype=np.float32)


def kernel(hidden_states, active_mask, uniform, W, b):
    hidden_states = np.asarray(hidden_states)
    active_mask = np.asarray(active_mask).astype(bool)
    uniform = np.asarray(uniform, dtype=np.float32)
    W = np.asarray(W, dtype=np.float32)
    b = np.asarray(b, dtype=np.float32)

    scores_flat = _device_scores(hidden_states, W)

    try:
        return _postprocess_jax(scores_flat, active_mask, uniform, b)
    except Exception:
        return _postprocess_np(scores_flat, active_mask, uniform, b)


# revision 34
# speedup vs baseline: 1.0353x; 1.0353x over previous
"""MoE router kernel for Trainium2 (raw Bass), 8-core data-parallel.

Problem (hardcoded shapes): hidden_states [4, 8192, 2048] f32, active_mask
[4, 8192] bool, uniform [4, 8192] f32, W [2048] f32, b () f32.

reference = router scores (matvec over D=2048) -> gumbel-noised global
top-k boolean mask + scalar aux loss.

Device work (the memory-bound part, 256 MiB read): scores = hs @ W,
sharded data-parallel over the 32768 tokens across 8 NeuronCores (4096
tokens / 32 MiB each), W replicated.  Per core: a tapered stream of
input DMAs (1 MiB chunks at the edges, 2 MiB steady state, dual HWDGE
rings, 16 MiB block-ring buffer) and 32 fused multiply+reduce ops on
DVE (scalar_tensor_tensor with accum_out) -> 4096 scores.  Measured
~89 us/core steady state (~375 GB/s/core effective), at the HBM-per-
core roofline.

Host work (O(B*S) = 32768 elements, negligible): gumbel noise, global
top-k threshold + mask, aux losses -- done with jax-on-CPU when
available (bit-identical op sequence to the reference), numpy fallback.
"""

import os
import sys

import numpy as np

for _p in ("/opt/trn_rl_repo", os.path.expanduser("~/.axon_site/_ro/trn_rl_repo")):
    if os.path.isdir(_p) and _p not in sys.path:
        sys.path.append(_p)

import concourse.bass as bass
import concourse.mybir as mybir
from concourse.bass_utils import run_bass_kernel_spmd

B, S, D = 4, 8192, 2048
T = B * S                  # 32768 tokens
NCORES = 8
TC = T // NCORES           # 4096 tokens per core
P = 128                    # SBUF partitions
NCOL = TC // P             # 32 score columns (128-token / 1 MiB blocks)
NBUFB = 16                 # h-buffer ring size in blocks (16 MiB)
# tapered DMA chunk schedule, in blocks: small first chunks so DVE starts
# sooner, small last chunks so the drain tail is short; 2 MiB steady-state
BLOCKS_PER_CHUNK = [1, 1] + [2] * 14 + [1, 1]  # sums to NCOL

CAPACITY = 0.55
TEMPERATURE = 1.0
LB_W, Z_W, ENT_W = 0.01, 1e-4, 1e-3

_BUILT = None
last_exec_time_ns = None
last_trace_path = None


def _build_bass(reps: int = 1):
    """Raw-bass SPMD program for one core: scores[t] = sum_d hs[t,d] * w[d].

    reps > 1 repeats the whole pipeline back-to-back inside the NEFF
    (identical work each rep) — used only for wall-clock benchmarking.

    Layout: token t = p*NCOL + c maps to partition p, 128-token (1 MiB)
    block c; blocks are grouped into DMA chunks per BLOCKS_PER_CHUNK
    (tapered: 1 MiB at the edges, 2 MiB steady state) landing in an
    NBUFB-block SBUF ring, then one fused multiply+reduce op
    (TensorScalarPtr w/ accum_out) on DVE per block produces one column
    of ssb [128, NCOL].

    Input DMAs alternate between the SP and ACT HWDGE rings so descriptor
    generation/completion overheads of consecutive transfers overlap; the
    tiny score store runs on GPSIMD (SWDGE) so the input rings never
    stall; the W broadcast heads the ACT ring while SP's chunk 0 (DVE's
    first need) starts in parallel.

    (TileContext is unusable with this compiler build: its end-of-kernel
    barrier emits multi-wait instructions that walrus rejects with "Too
    many sync wait commands", so synchronization is explicit here.)
    """
    blocks = BLOCKS_PER_CHUNK
    nchunk = len(blocks)
    gb_start = [0]
    for k in blocks[:-1]:
        gb_start.append(gb_start[-1] + k)
    # no chunk may wrap the block ring, and reps must realign the ring
    assert sum(blocks) == NCOL and NCOL % NBUFB == 0
    for s, k in zip(gb_start, blocks):
        assert (s % NBUFB) + k <= NBUFB, (s, k)

    nc = bass.Bass(trn_type="TRN2")
    hs = nc.dram_tensor("hs", [TC, D], mybir.dt.float32, kind="ExternalInput")
    w = nc.dram_tensor("w", [D], mybir.dt.float32, kind="ExternalInput")
    scores = nc.dram_tensor("scores", [TC], mybir.dt.float32, kind="ExternalOutput")

    # token t = p*NCOL + c: per-partition contiguous input runs and a
    # contiguous [128, NCOL] score store.
    hs_rr = hs[:].rearrange("(p c) d -> p c d", p=P)     # [128, NCOL, D]
    scores_r = scores[:].rearrange("(p c) -> p c", p=P)  # [128, NCOL]

    rings = [[i for i in range(nchunk) if i % 2 == k] for k in range(2)]
    ring_of = {i: i % 2 for i in range(nchunk)}
    ring_ord = {}
    for k in range(2):
        for o, i in enumerate(rings[k]):
            ring_ord[i] = o
    RING_INC = [16 * len(rings[k]) for k in range(2)]
    # each ring carries half the W broadcast up front (completes in half
    # the time and overlaps the first input chunk)
    V_INC = NCOL  # v_sem counts completed blocks

    with (
        nc.sbuf_tensor([P, D], mybir.dt.float32) as wb,
        nc.sbuf_tensor([P, NBUFB * D], mybir.dt.float32) as hbuf,
        # double-buffered by rep parity so rep r+1's muls never wait on
        # rep r's out-DMA (reps>1 benchmarking only; reps=1 uses half)
        nc.sbuf_tensor([P, 2 * NCOL], mybir.dt.float32) as ssb,
        nc.sbuf_tensor([P, 1], mybir.dt.float32) as dummy,
        nc.semaphore() as sem_sp,
        nc.semaphore() as sem_act,
        nc.semaphore() as sem_pool,
        nc.semaphore() as v_sem,
        # gp's explicit sem_pool wait already proves the score store
        # landed; skip GPSIMD's expensive SWDGE dge_drain at the exit
        # barrier.
        nc.Block(no_gpsimd_drain=True) as block,
    ):
        ring_sems = [sem_sp, sem_act]

        def ssb_half(r):
            base = (r % 2) * NCOL
            return ssb[:, base:base + NCOL]

        def input_stream(eng, k):
            sem = ring_sems[k]
            half = D // 2
            lo, hi = k * half, (k + 1) * half
            eng.dma_start(
                wb[:, lo:hi], w[:][lo:hi].unsqueeze(0).broadcast_to([P, half])
            ).then_inc(sem, 16)
            for r in range(reps):
                for i in rings[k]:
                    kblk = blocks[i]
                    gb = r * NCOL + gb_start[i]  # global block index
                    # ring slot gb%NBUFB is free once the mul of block
                    # gb+kblk-NBUFB has run (v_sem counts muls/blocks)
                    free_at = gb + kblk - NBUFB
                    if free_at > 0:
                        eng.wait_ge(v_sem, free_at)
                    s = gb % NBUFB
                    eng.dma_start(
                        hbuf[:, s * D:(s + kblk) * D],
                        hs_rr[:, gb_start[i]:gb_start[i] + kblk, :],
                    ).then_inc(sem, 16)

        @block.sync
        def _(sync):
            # half of W, then even chunks
            input_stream(sync, 0)
            if reps == 1:
                # single-shot: final store on the now-idle SP HWDGE ring
                # (cheaper fixed cost than SWDGE; GPSIMD stays unused)
                sync.wait_ge(v_sem, V_INC)
                sync.dma_start(scores_r, ssb_half(0)).then_inc(sem_sp, 16)
                sync.wait_ge(sem_sp, 16 * (len(rings[0]) + 2))  # store landed

        @block.scalar
        def _(scalar):
            # other half of W, then odd chunks
            input_stream(scalar, 1)

        if reps > 1:
            @block.gpsimd
            def _(gp):
                # tiny per-rep score store off the input rings so they
                # never stall at rep boundaries
                for r in range(reps):
                    gp.wait_ge(v_sem, (r + 1) * V_INC)
                    gp.dma_start(scores_r, ssb_half(r)).then_inc(sem_pool, 16)
                gp.wait_ge(sem_pool, 16 * reps)  # final out-DMA landed

        @block.vector
        def _(vector):
            for r in range(reps):
                if r == 0:
                    # ACT's wb half; SP's half is subsumed by chunk 0's wait
                    vector.wait_ge(sem_act, 16)
                elif r >= 2:
                    # rep r-2's out-DMA done (WAR on this parity's ssb)
                    vector.wait_ge(sem_pool, 16 * (r - 1))
                for i in range(nchunk):
                    k = ring_of[i]
                    vector.wait_ge(
                        ring_sems[k],
                        r * RING_INC[k] + 16 + 16 * (ring_ord[i] + 1),
                    )
                    for b in range(blocks[i]):
                        col = gb_start[i] + b
                        s = (r * NCOL + col) % NBUFB
                        nc.vector.scalar_tensor_tensor(
                            dummy[:].broadcast_to([P, D]),
                            hbuf[:, s * D:(s + 1) * D],
                            1.0,
                            wb[:],
                            op0=mybir.AluOpType.bypass,
                            op1=mybir.AluOpType.mult,
                            accum_out=ssb_half(r)[:, col:col + 1],
                        ).then_inc(v_sem, 1)
    return nc


def _device_scores(hidden_states, W):
    """Run the sharded matvec on 8 NeuronCores; returns scores [T] f32."""
    global _BUILT, last_exec_time_ns, last_trace_path
    if _BUILT is None:
        _BUILT = _build_bass()
    nc = _BUILT
    hs_flat = np.ascontiguousarray(
        np.asarray(hidden_states, dtype=np.float32).reshape(T, D)
    )
    w_np = np.ascontiguousarray(np.asarray(W, dtype=np.float32).reshape(D))
    in_maps = [
        {"hs": hs_flat[c * TC:(c + 1) * TC], "w": w_np} for c in range(NCORES)
    ]
    res = run_bass_kernel_spmd(nc, in_maps, core_ids=list(range(NCORES)))
    last_exec_time_ns = res.exec_time_ns
    if res.instructions_and_trace is not None:
        last_trace_path = res.instructions_and_trace[1]
    return np.concatenate([res.results[c]["scores"] for c in range(NCORES)])


def _postprocess_np(scores_flat, active_mask, uniform, b):
    """Numpy fallback for the O(T) post-processing (f32-faithful)."""
    f32 = np.float32
    scores = scores_flat.reshape(B, S) + f32(b)
    scores = np.where(active_mask, scores, f32(-1e9)).astype(np.float32)
    k = max(1, min(int(CAPACITY * T + 0.5), T))
    u = np.clip(uniform.astype(np.float32), f32(1e-6), f32(1.0 - 1e-6))
    gumbel = np.clip(
        -np.log(-np.log(u) + f32(1e-6)), f32(-10.0), f32(10.0)
    ).astype(np.float32)
    noisy = (scores + gumbel * f32(TEMPERATURE)).astype(np.float32)
    flat = noisy.reshape(-1)
    kth = np.partition(flat, T - k)[T - k]
    mask_flat = flat > kth
    n_gt = int(mask_flat.sum())
    if n_gt < k:  # fill ties at the threshold in index order (lax.top_k is stable)
        ties = np.nonzero(flat == kth)[0]
        mask_flat[ties[: k - n_gt]] = True
    ffn_mask = mask_flat.reshape(B, S) & active_mask

    with np.errstate(over="ignore"):
        probs = np.where(
            scores >= 0,
            f32(1.0) / (f32(1.0) + np.exp(-scores)),
            np.exp(scores) / (f32(1.0) + np.exp(scores)),
        ).astype(np.float32)
    lb_loss = (probs.mean(dtype=np.float32) - f32(CAPACITY)) ** 2
    z_loss = (scores.astype(np.float32) ** 2).mean(dtype=np.float32)
    p = np.clip(probs, f32(1e-4), f32(1.0 - 1e-4))
    entropy = -(
        p * np.log(p) + (f32(1.0) - p) * np.log(f32(1.0) - p)
    ).mean(dtype=np.float32)
    aux = f32(LB_W) * lb_loss + f32(Z_W) * z_loss + f32(ENT_W) * (-entropy)
    return np.asarray(ffn_mask, dtype=bool), np.float32(aux)


def _postprocess_jax(scores_flat, active_mask, uniform, b):
    """Reference-identical op sequence on jax CPU (given device scores)."""
    import jax
    import jax.numpy as jnp

    cpu = jax.devices("cpu")[0]
    with jax.default_device(cpu):
        scores = jnp.asarray(scores_flat, dtype=jnp.float32).reshape(B, S) + jnp.asarray(
            b, dtype=jnp.float32
        )
        am = jnp.asarray(active_mask)
        scores = jnp.where(am, scores, jnp.float32(-1e9))
        k = max(1, min(int(CAPACITY * T + 0.5), T))
        u = jnp.clip(jnp.asarray(uniform, dtype=jnp.float32), 1e-6, 1.0 - 1e-6)
        gumbel = jnp.clip(-jnp.log(-jnp.log(u) + 1e-6), -10.0, 10.0)
        noisy = scores + gumbel * TEMPERATURE
        flat = noisy.reshape(-1)
        _, idx = jax.lax.top_k(flat, k)
        ffn_mask = jnp.zeros((T,), dtype=bool).at[idx].set(True)
        ffn_mask = ffn_mask.reshape(B, S) & am

        probs = jax.nn.sigmoid(scores)
        lb_loss = (probs.mean() - jnp.float32(CAPACITY)) ** 2
        z_loss = (scores ** 2).mean()
        p = jnp.clip(probs, 1e-4, 1.0 - 1e-4)
        entropy = -(p * jnp.log(p) + (1.0 - p) * jnp.log(1.0 - p)).mean()
        aux = LB_W * lb_loss + Z_W * z_loss + ENT_W * (-entropy)
        return np.asarray(ffn_mask), np.asarray(aux, dtype=np.float32)


def kernel(hidden_states, active_mask, uniform, W, b):
    hidden_states = np.asarray(hidden_states)
    active_mask = np.asarray(active_mask).astype(bool)
    uniform = np.asarray(uniform, dtype=np.float32)
    W = np.asarray(W, dtype=np.float32)
    b = np.asarray(b, dtype=np.float32)

    scores_flat = _device_scores(hidden_states, W)

    try:
        return _postprocess_jax(scores_flat, active_mask, uniform, b)
    except Exception:
        return _postprocess_np(scores_flat, active_mask, uniform, b)
